# revision 1
# baseline (speedup 1.0000x reference)
"""DNADecoder TRN2 kernel: 2-core ping-pong decode chain.

Core 0 (of the shared-scratchpad pair {0,1}): emb gather + layers 0-1.
Core 1: layers 2-3 + streamed out_w logits + argmax + softmax + probs export.
Handoff via pair-shared DRAM scratchpad with nonce flags + register polling.

Host precomputes (numpy, cheap, one-time): cross-attention collapse
(ca_out[l] = (mem1@Wv+bv)@Wo+bo, exact because memory rows are identical
across T so attention weights are uniform), self-attention fusion
(W_sa = Wv@Wo), and weight block packing.

Everything on-device is fp32, feature-major ("xT") activations with
weight-stationary matmuls (zero transposes in the chain).
"""

import numpy as np

B, T_FULL, P_IN, D, F, V, L, H = 64, 128, 1024, 512, 2048, 4096, 4, 8
NJ = D // 128            # 4 feature blocks of x
XC = NJ * B // 1         # 256 cols of xT layout [128, NJ*64]
NK1, NM1 = D // 128, F // 128    # ffn1 blocks 4x16
NK2, NM2 = F // 128, D // 128    # ffn2 blocks 16x4
NSA = D // 128                   # sa blocks 4x4
SEQ0 = [(0, "sa"), (0, "w1"), (0, "w2"), (1, "sa"), (1, "w1"), (1, "w2")]
SEQ1 = [(2, "sa"), (2, "w1"), (2, "w2"), (3, "sa"), (3, "w1"), (3, "w2")]
NTILES = 288             # per role
WCOLS = NTILES * 128
NCHUNK = 8               # out_w chunks of [512, 512]
EPS = 1e-5


def _shapes(mat):
    return {"sa": (4, 4), "w1": (4, 16), "w2": (16, 4)}[mat]


def build_program(T, lite=False):
    import concourse.bass as bass
    import concourse.mybir as mybir
    from concourse import bacc
    from concourse.bass import ds

    f32 = mybir.dt.float32
    i32 = mybir.dt.int32
    u32 = mybir.dt.uint32
    A = mybir.AluOpType
    AF = mybir.ActivationFunctionType

    nc = bacc.Bacc("TRN2", target_bir_lowering=False)

    # ---- I/O ----
    inp = {}
    for name, shape, dt in [
        ("wblk", [128, WCOLS], f32), ("pext", [128, 4 * T], f32),
        ("lnp", [128, 96], f32), ("bsa", [128, 16], f32),
        ("b1x", [128, 64], f32), ("b2x", [128, 16], f32),
        ("caxt", [128, 1024], f32), ("consts", [128, 130], f32),
        ("crow", [1, 128], f32), ("outb", [8, 512], f32),
        ("tok0", [64, 1], i32), ("noncei", [1, 1], i32),
        ("emb_hbm", [V, D], f32), ("wout_hbm", [4 * NCHUNK, 128, 512], f32),
    ]:
        inp[name] = nc.declare_dram_parameter(name, shape, dt, isOutput=False)
    probs_out = nc.declare_dram_parameter(
        "probs_out", [B, 1 if lite else T, V], f32, isOutput=True)
    toks_out = nc.declare_dram_parameter("toks_out", [T, B], i32, isOutput=True)

    # ---- pair-shared mailboxes ----
    mail = nc.dram_tensor("mail", [128, T * XC], f32, addr_space="Shared")
    probs_int = nc.dram_tensor("probs_int", [B, T, V], f32)
    emb_int = nc.dram_tensor("emb_int", [V, D], f32)
    toks_int = nc.dram_tensor("toks_int", [T, 64], i32)
    tokm = nc.dram_tensor("tokm", [T, 64], i32, addr_space="Shared")
    flag_m = nc.dram_tensor("flag_m", [1, T], i32, addr_space="Shared")
    flag_t = nc.dram_tensor("flag_t", [1, T], i32, addr_space="Shared")

    ctxs = []

    def sb(name, shape, dt=f32):
        cm = nc.sbuf_tensor(name, shape, dt)
        h = cm.__enter__()
        ctxs.append(cm)
        return h

    def psum(name, shape):
        cm = nc.psum_tensor(name, shape, f32)
        h = cm.__enter__()
        ctxs.append(cm)
        return h

    wblk_sb = sb("wblk_sb", [128, WCOLS])
    pext_sb = sb("pext_sb", [128, 4 * T])
    lnp_sb = sb("lnp_sb", [128, 96])
    bsa_sb = sb("bsa_sb", [128, 16])
    b1x_sb = sb("b1x_sb", [128, 64])
    b2x_sb = sb("b2x_sb", [128, 16])
    caxt_sb = sb("caxt_sb", [128, 1024])
    consts_sb = sb("consts_sb", [128, 130])
    crow_sb = sb("crow_sb", [1, 128])
    E_sb = sb("E_sb", [64, 512])
    pe_cur = sb("pe_cur", [128, 4])
    x_sb = sb("x_sb", [128, XC])
    y_sb = sb("y_sb", [128, XC])
    tmp_sb = sb("tmp_sb", [128, XC])
    sq_sb = sb("sq_sb", [128, XC])
    h_sb = sb("h_sb", [128, 1024])
    stats_sb = sb("stats_sb", [1, 128])
    msq_sb = sb("msq_sb", [1, 64])
    var_sb = sb("var_sb", [1, 64])
    rstd_sb = sb("rstd_sb", [1, 64])
    logits_sb = sb("logits_sb", [64, 4096])
    mx_sb = sb("mx_sb", [64, 8])
    idx_sb = sb("idx_sb", [64, 8], u32)
    tok_sb = sb("tok_sb", [64, 1], i32)
    negmx_sb = sb("negmx_sb", [64, 1])
    sums_sb = sb("sums_sb", [64, 1])
    rec_sb = sb("rec_sb", [64, 1])
    st_sb = sb("st_sb", [128, 5120])
    nsb = sb("nsb", [1, 2], i32)
    fsb = sb("fsb", [1, 2], i32)
    tki = sb("tki", [64, 1], i32)

    ps_big = psum("ps_big", [128, 1024])
    ps_tr = psum("ps_tr", [128, 256])
    ps_misc = psum("ps_misc", [128, 512])
    ps_log = psum("ps_log", [64, 1024])

    ident = consts_sb[:, 0:128]
    sc512 = consts_sb[:, 128:129]       # 1/512 column
    onesr = crow_sb[0:1, 0:128]
    ones64 = crow_sb[0:1, 0:64]

    SEMS = ["s_g", "s_p", "s_v", "s_a", "s_st", "s_cons", "s_pd", "s_dg"]
    sem_h = {}
    for s in SEMS:
        cm = nc.semaphore(s)
        sem_h[s] = cm.__enter__()
        ctxs.append(cm)

    # ---------------- schedule builder ----------------
    # op = (eng, emit_fn_or_None_builder, waits, signals)
    # waits: [sem], signals: [(sem, amt)]
    def wcol(role, mm_i, m, k):
        base = 0
        seq = SEQ0 if role == 0 else SEQ1
        for i in range(mm_i):
            nk, nm = _shapes(seq[i][1])
            base += nk * nm
        nk, nm = _shapes(seq[mm_i][1])
        return (base + m * nk + k) * 128

    def build_sched(role, t, dyn, geng):
        """t: python int for peeled step 0, else None (use dyn regs).
        dyn: dict of ScalarValues (only valid when walking engine 'g').
        geng: the gpsimd engine object (for g ops), or None."""
        ops = []

        def op(eng, emit, waits=(), signals=()):
            ops.append((eng, emit, tuple(waits), tuple(signals)))

        def gdma(out, in_, indirect=None):
            # emit dma on g + bump cnt reg; completion waited via s_dg cnt
            g = geng
            if indirect is None:
                ins = g.dma_start(out=out, in_=in_)
            else:
                ins = g.indirect_dma_start(
                    out=out, out_offset=None, in_=in_,
                    in_offset=bass.IndirectOffsetOnAxis(ap=indirect, axis=0),
                )
            ins.then_inc(sem_h["s_dg"], 16)
            g.reg_add(dyn["cnt"], dyn["cnt"], 16)
            return ins

        def gwait():
            geng.wait_ge(sem_h["s_dg"], dyn["cnt"])

        def emit_ln(lidx, ln_i, ysem):
            # LN over y_sb -> x_sb; ysem: sem of y's last producer
            gb = lambda gb_i, j: lnp_sb[:, lidx * 24 + ln_i * 8 + gb_i * 4 + j:
                                        lidx * 24 + ln_i * 8 + gb_i * 4 + j + 1]

            def esq(e):
                return e.activation(out=sq_sb[:, :], in_=y_sb[:, :], func=AF.Square)
            op("a", esq, waits=[ysem], signals=[("s_a", 1)])

            def esums(e):
                ii = None
                for j in range(NJ):
                    ii = e.matmul(out=ps_misc[0:1, 0:64], lhsT=sc512,
                                  rhs=y_sb[:, j * 64:(j + 1) * 64],
                                  start=(j == 0), stop=(j == NJ - 1))
                for j in range(NJ):
                    ii = e.matmul(out=ps_misc[0:1, 64:128], lhsT=sc512,
                                  rhs=sq_sb[:, j * 64:(j + 1) * 64],
                                  start=(j == 0), stop=(j == NJ - 1))
                return ii
            op("p", esums, waits=["s_a"], signals=[("s_p", 1)])

            op("a", lambda e: e.activation(out=stats_sb[:, :], in_=ps_misc[0:1, 0:128],
                                           func=AF.Copy),
               waits=["s_p"], signals=[("s_a", 1)])

            def evar(e):
                e.tensor_tensor(out=msq_sb[:, :], in0=stats_sb[0:1, 0:64],
                                in1=stats_sb[0:1, 0:64], op=A.mult)
                e.drain()
                e.tensor_tensor(out=var_sb[:, :], in0=stats_sb[0:1, 64:128],
                                in1=msq_sb[:, :], op=A.subtract)
                e.drain()
                return e.tensor_scalar(out=var_sb[:, :], in0=var_sb[:, :],
                                       scalar1=consts_sb[0:1, 129:130],
                                       scalar2=None, op0=A.add)
            op("v", evar, waits=["s_a"], signals=[("s_v", 1)])

            op("a", lambda e: e.activation(out=var_sb[:, :], in_=var_sb[:, :],
                                           func=AF.Sqrt),
               waits=["s_v"], signals=[("s_a", 1)])
            op("v", lambda e: e.reciprocal(out=rstd_sb[:, :], in_=var_sb[:, :]),
               waits=["s_a"], signals=[("s_v", 1)])

            def ebc(e):
                e.matmul(out=ps_misc[:, 128:192], lhsT=onesr,
                         rhs=stats_sb[0:1, 0:64], start=True, stop=True)
                return e.matmul(out=ps_misc[:, 192:256], lhsT=onesr,
                                rhs=rstd_sb[0:1, 0:64], start=True, stop=True)
            op("p", ebc, waits=["s_v"], signals=[("s_p", 1)])

            def enrm(e):
                ii = None
                for j in range(NJ):
                    ii = e.tensor_tensor(out=tmp_sb[:, j * 64:(j + 1) * 64],
                                         in0=y_sb[:, j * 64:(j + 1) * 64],
                                         in1=ps_misc[:, 128:192], op=A.subtract)
                e.drain()
                for j in range(NJ):
                    ii = e.tensor_tensor(out=tmp_sb[:, j * 64:(j + 1) * 64],
                                         in0=tmp_sb[:, j * 64:(j + 1) * 64],
                                         in1=ps_misc[:, 192:256], op=A.mult)
                return ii
            op("v", enrm, waits=["s_p"], signals=[("s_v", 1)])

            def eaff(e):
                e.drain()
                ii = None
                for j in range(NJ):
                    ii = e.tensor_scalar(out=x_sb[:, j * 64:(j + 1) * 64],
                                         in0=tmp_sb[:, j * 64:(j + 1) * 64],
                                         scalar1=gb(0, j), scalar2=gb(1, j),
                                         op0=A.mult, op1=A.add)
                return ii
            op("v", eaff, waits=["s_p"], signals=[("s_v", 1)])

        def emit_wsmm(mm_i, role, mat, xsem, dst, dcol0):
            # weight-stationary matmul: out-blocks accumulated over k
            nk, nm = _shapes(mat)
            src = h_sb if mat == "w2" else x_sb

            def emm(e):
                ii = None
                for m in range(nm):
                    for k in range(nk):
                        ii = e.matmul(
                            out=dst[:, dcol0 + m * 64:dcol0 + (m + 1) * 64],
                            lhsT=wblk_sb[:, wcol(role, mm_i, m, k):
                                         wcol(role, mm_i, m, k) + 128],
                            rhs=src[:, k * 64:(k + 1) * 64],
                            start=(k == 0), stop=(k == nk - 1))
                return ii
            op("p", emm, waits=[xsem], signals=[("s_p", 1)])

        def emit_layer(role, lidx, mm_base, xsem):
            # SA
            emit_wsmm(mm_base + 0, role, "sa", xsem, ps_big, 0)

            def eres_sa(e):
                e.drain()
                return e.tensor_tensor(out=y_sb[:, :], in0=x_sb[:, :],
                                       in1=ps_big[:, 0:XC], op=A.add)
            op("v", eres_sa, waits=["s_p"], signals=[("s_v", 1)])

            def ebias_sa(e):
                e.drain()
                ii = None
                for j in range(NJ):
                    ii = e.tensor_scalar(out=y_sb[:, j * 64:(j + 1) * 64],
                                         in0=y_sb[:, j * 64:(j + 1) * 64],
                                         scalar1=bsa_sb[:, lidx * 4 + j:
                                                        lidx * 4 + j + 1],
                                         scalar2=None, op0=A.add)
                return ii
            op("v", ebias_sa, waits=[], signals=[("s_v", 1)])
            emit_ln(lidx, 0, "s_v")

            # CA add (x + ca_out)
            def eca(e):
                e.drain()
                return e.tensor_tensor(out=y_sb[:, :], in0=x_sb[:, :],
                                       in1=caxt_sb[:, lidx * 256:(lidx + 1) * 256],
                                       op=A.add)
            op("v", eca, waits=[], signals=[("s_v", 1)])
            emit_ln(lidx, 1, "s_v")

            # FFN1 + relu
            emit_wsmm(mm_base + 1, role, "w1", "s_v", ps_big, 0)

            def erelu(e):
                ii = None
                for m in range(NM1):
                    ii = e.activation(out=h_sb[:, m * 64:(m + 1) * 64],
                                      in_=ps_big[:, m * 64:(m + 1) * 64],
                                      func=AF.Relu,
                                      bias=b1x_sb[:, lidx * 16 + m:lidx * 16 + m + 1])
                return ii
            op("a", erelu, waits=["s_p"], signals=[("s_a", 1)])

            # FFN2 + residual + bias
            emit_wsmm(mm_base + 2, role, "w2", "s_a", ps_big, 0)

            def eres_f(e):
                e.drain()
                return e.tensor_tensor(out=y_sb[:, :], in0=x_sb[:, :],
                                       in1=ps_big[:, 0:XC], op=A.add)
            op("v", eres_f, waits=["s_p"], signals=[("s_v", 1)])

            def ebias_f(e):
                e.drain()
                ii = None
                for j in range(NJ):
                    ii = e.tensor_scalar(out=y_sb[:, j * 64:(j + 1) * 64],
                                         in0=y_sb[:, j * 64:(j + 1) * 64],
                                         scalar1=b2x_sb[:, lidx * 4 + j:
                                                        lidx * 4 + j + 1],
                                         scalar2=None, op0=A.add)
                return ii
            op("v", ebias_f, waits=[], signals=[("s_v", 1)])
            emit_ln(lidx, 2, "s_v")

        # ================= role 0 =================
        if role == 0:
            def eg_in(e):
                if t is None:
                    # poll token flag of step t-1, fetch token
                    uid = nc.next_id()
                    with e.register(f"fv0_{uid}") as fv, \
                         e.register(f"pc0_{uid}") as pc:
                        e.reg_mov(fv, 0)

                        def cond():
                            e.reg_alu(pc, fv, dyn["nonce"], A.not_equal)
                            return pc
                        with e.While(cond):
                            gdma(fsb[:, 0:1], flag_t[0:1, ds(dyn["tm1"], 1)])
                            gwait()
                            e.reg_load(fv, fsb[0:1, 0:1])
                        gdma(tki[:, :], tokm[ds(dyn["tm1"], 1), :])
                        gwait()
                gdma(E_sb[:, :], emb_int[:, :], indirect=tki[:, 0:1])
                gdma(pe_cur[:, :], pext_sb[:, ds(dyn["t4"], 4)]
                     if t is None else pext_sb[:, 4 * t:4 * t + 4])
                gwait()
                return e.nop()
            op("g", eg_in, waits=["s_a", "s_p", "s_v"], signals=[("s_g", 1)])

            def etr(e):
                ii = None
                for j in range(NJ):
                    ii = e.transpose(out=ps_tr[:, j * 64:(j + 1) * 64],
                                     in_=E_sb[:, j * 128:(j + 1) * 128],
                                     identity=ident[0:64, 0:64])
                return ii
            op("p", etr, waits=["s_g"], signals=[("s_p", 1)])

            def ex0(e):
                e.drain()
                ii = None
                for j in range(NJ):
                    ii = e.tensor_scalar(out=x_sb[:, j * 64:(j + 1) * 64],
                                         in0=ps_tr[:, j * 64:(j + 1) * 64],
                                         scalar1=pe_cur[:, j:j + 1], scalar2=None,
                                         op0=A.add)
                return ii
            op("v", ex0, waits=["s_p"], signals=[("s_v", 1)])

            emit_layer(0, 0, 0, "s_v")
            emit_layer(0, 1, 3, "s_v")

            def eg_send(e):
                gdma(mail[:, ds(dyn["txc"], XC)] if t is None
                     else mail[:, XC * t:XC * (t + 1)], x_sb[:, :])
                gwait()
                gdma(flag_m[0:1, ds(dyn["t"], 1)] if t is None
                     else flag_m[0:1, t:t + 1], nsb[:, 0:1])
                gwait()
                return e.nop()
            op("g", eg_send, waits=["s_v"], signals=())

        # ================= role 1 =================
        else:
            def eg_in(e):
                uid = nc.next_id()
                with e.register(f"fv1_{uid}") as fv, \
                     e.register(f"pc1_{uid}") as pc:
                    e.reg_mov(fv, 0)

                    def cond():
                        e.reg_alu(pc, fv, dyn["nonce"], A.not_equal)
                        return pc
                    with e.While(cond):
                        gdma(fsb[:, 0:1], flag_m[0:1, ds(dyn["t"], 1)]
                             if t is None else flag_m[0:1, t:t + 1])
                        gwait()
                        e.reg_load(fv, fsb[0:1, 0:1])
                gdma(x_sb[:, :], mail[:, ds(dyn["txc"], XC)]
                     if t is None else mail[:, XC * t:XC * (t + 1)])
                gwait()
                return e.nop()
            op("g", eg_in, waits=["s_p", "s_a", "s_v"], signals=[("s_g", 1)])
            emit_layer(1, 2, 0, "s_g")
            emit_layer(1, 3, 3, "s_v")

            # ---- logits: stream out_w, act-stationary ----
            fills = []
            cons = []
            for n in range(NCHUNK):
                def efill(e, n=n):
                    for k in range(4):
                        gdma(st_sb[:, (n % 2) * 2560 + k * 512:
                                   (n % 2) * 2560 + (k + 1) * 512],
                             inp["wout_hbm"][4 * n + k, :, :])
                    gdma(st_sb[0:1, (n % 2) * 2560 + 2048:
                               (n % 2) * 2560 + 2560],
                         inp["outb"][n:n + 1, :])
                    gwait()
                    return e.nop()
                fills.append((efill, ["s_a"] if n >= 2 else []))

                def econs(e, n=n):
                    sl = (n % 2)
                    for k in range(4):
                        e.matmul(out=ps_log[:, sl * 512:(sl + 1) * 512],
                                 lhsT=x_sb[:, k * 64:(k + 1) * 64],
                                 rhs=st_sb[:, sl * 2560 + k * 512:
                                           sl * 2560 + (k + 1) * 512],
                                 start=(k == 0), stop=False)
                    return e.matmul(out=ps_log[:, sl * 512:(sl + 1) * 512],
                                    lhsT=ones64,
                                    rhs=st_sb[0:1, sl * 2560 + 2048:
                                              sl * 2560 + 2560],
                                    start=False, stop=True)
                cons.append(econs)

            # interleave: f0 f1 c0 f2 c1 f3 c2 ... f7 c6 c7
            order = []
            fi, ci = 0, 0
            while fi < NCHUNK or ci < NCHUNK:
                if fi < NCHUNK and fi <= ci + 1:
                    order.append(("f", fi)); fi += 1
                else:
                    order.append(("c", ci)); ci += 1
            for kind, n in order:
                if kind == "f":
                    op("g", fills[n][0], waits=fills[n][1], signals=[("s_st", 1)])
                else:
                    op("p", cons[n], waits=["s_st", "s_a"] + (["s_v"] if n == 0
                                                               else []),
                       signals=[("s_p", 1)])

                    def ecp(e, n=n):
                        return e.activation(out=logits_sb[:, n * 512:(n + 1) * 512],
                                            in_=ps_log[:, (n % 2) * 512:
                                                       ((n % 2) + 1) * 512],
                                            func=AF.Copy)
                    op("a", ecp, waits=["s_p"] + (["s_pd"] if n == 0 else []),
                       signals=[("s_a", 1)])

            def eargm(e):
                e.max(out=mx_sb[:, :], in_=logits_sb[:, :])
                e.drain()
                e.max_index(out=idx_sb[:, :], in_max=mx_sb[:, :],
                            in_values=logits_sb[:, :])
                e.drain()
                return e.tensor_copy(out=tok_sb[:, :], in_=idx_sb[:, 0:1])
            op("v", eargm, waits=["s_a"], signals=[("s_v", 1)])

            def eg_tok(e):
                gdma(tokm[ds(dyn["t"], 1), :] if t is None else tokm[t:t + 1, :],
                     tok_sb[:, :])
                gwait()
                gdma(flag_t[0:1, ds(dyn["t"], 1)] if t is None
                     else flag_t[0:1, t:t + 1], nsb[:, 0:1])
                gdma(toks_int[ds(dyn["t"], 1), :] if t is None
                     else toks_int[t:t + 1, :], tok_sb[:, :])
                gwait()
                return e.nop()
            op("g", eg_tok, waits=["s_v"], signals=())

            # softmax (off critical path)
            def enmx(e):
                e.drain()
                return e.tensor_scalar(out=negmx_sb[:, :], in0=mx_sb[:, 0:1],
                                       scalar1=-1.0, scalar2=None, op0=A.mult)
            op("v", enmx, waits=[], signals=[("s_v", 1)])

            def eexp(e):
                return e.activation(out=logits_sb[:, :], in_=logits_sb[:, :],
                                    func=AF.Exp, bias=negmx_sb[:, 0:1],
                                    accum_out=sums_sb[:, :])
            op("a", eexp, waits=["s_v", "s_pd"], signals=[("s_a", 1)])
            op("v", lambda e: e.reciprocal(out=rec_sb[:, :], in_=sums_sb[:, :]),
               waits=["s_a"], signals=[("s_v", 1)])

            def escale(e):
                return e.activation(out=logits_sb[:, :], in_=logits_sb[:, :],
                                    func=AF.Copy, scale=rec_sb[:, 0:1])
            op("a", escale, waits=["s_v", "s_a"], signals=[("s_a", 1)])

            def eg_probs(e):
                ins = e.dma_start(out=probs_int[:, dyn["t"] if t is None else t, :],
                                  in_=logits_sb[:, :])
                ins.then_inc(sem_h["s_pd"], 16)
                return e.nop()
            op("g", eg_probs, waits=["s_a"], signals=[("s_pd", 16)])

        return ops

    # ---------------- walker ----------------
    class Walker:
        def __init__(self, eng_name, eng):
            self.en = eng_name
            self.e = eng
            self.counts = {s: 0 for s in SEMS}
            self.last = {}
            self.regs = {}

        def prealloc(self, scheds):
            need = set()
            for sched in scheds:
                for eng, _, waits, _ in sched:
                    if eng == self.en:
                        for s in waits:
                            need.add(s)
            for s in sorted(need):
                r = self.e.alloc_register(f"thr_{self.en}_{s}")
                self.e.reg_mov(r, 0)
                self.regs[s] = r
                self.last[s] = 0

        def walk(self, sched, emit=True):
            for eng, fn, waits, signals in sched:
                if eng == self.en:
                    for s in waits:
                        delta = self.counts[s] - self.last[s]
                        if delta > 0:
                            self.e.reg_add(self.regs[s], self.regs[s], delta)
                            self.last[s] = self.counts[s]
                        self.e.wait_ge(sem_h[s], self.regs[s])
                    ins = fn(self.e)
                    first = True
                    for s, amt in signals:
                        if s == "s_pd":  # incremented by its dma directly
                            continue
                        if first:
                            ins.then_inc(sem_h[s], amt)
                            first = False
                        else:
                            self.e.nop().then_inc(sem_h[s], amt)
                for s, amt in signals:
                    self.counts[s] += amt

        def flush(self):
            # align threshold regs to full-iteration totals for loop back-edge
            for s, r in self.regs.items():
                delta = self.counts[s] - self.last[s]
                if delta > 0:
                    self.e.reg_add(r, r, delta)
                    self.last[s] = self.counts[s]

    def trace_engine(eng_name, eng):
        pid = eng.partition_id()
        role_r = eng.alloc_register(f"role_{eng_name}")
        eng.reg_alu(role_r, pid, 1, A.bitwise_and)

        # init DMAs on g (both roles identical)
        if eng_name == "g":
            cnt0 = eng.alloc_register("cnt_init")
            eng.reg_mov(cnt0, 0)
            nonce_r = eng.alloc_register("nonce_r")
            c = 0
            for dst, src in [
                (wblk_sb[:, :], inp["wblk"][:, :]),
                (pext_sb[:, :], inp["pext"][:, :]),
                (lnp_sb[:, :], inp["lnp"][:, :]),
                (bsa_sb[:, :], inp["bsa"][:, :]),
                (b1x_sb[:, :], inp["b1x"][:, :]),
                (b2x_sb[:, :], inp["b2x"][:, :]),
                (caxt_sb[:, :], inp["caxt"][:, :]),
                (consts_sb[:, :], inp["consts"][:, :]),
                (crow_sb[:, :], inp["crow"][:, :]),
                (tki[:, :], inp["tok0"][:, :]),
                (emb_int[:, :], inp["emb_hbm"][:, :]),
                (nsb[:, 0:1], inp["noncei"][:, :]),
            ]:
                eng.dma_start(out=dst, in_=src).then_inc(sem_h["s_dg"], 16)
                c += 16
            eng.reg_add(cnt0, cnt0, c)
            eng.wait_ge(sem_h["s_dg"], cnt0)
            eng.reg_load(nonce_r, nsb[0:1, 0:1])
            init_sig = eng.nop()
            init_sig.then_inc(sem_h["s_g"], 1)
        w = Walker(eng_name, eng)

        for role in (0, 1):
            # dyn values builder for g inside loop
            def make_dyn(iv):
                d = {}
                if eng_name != "g":
                    return d
                for key, mulv in [("t", 1), ("tm1", 1), ("t4", 4), ("txc", XC)]:
                    r = eng.alloc_register(f"dyn_{role}_{key}")
                    base = iv if key != "tm1" else None
                    if key == "tm1":
                        eng.reg_alu(r, iv, 1, A.subtract)
                    else:
                        eng.reg_alu(r, iv, mulv, A.mult)
                    d[key] = eng.snap(r, min_val=0, max_val=(T - 1) * mulv)
                d["nonce"] = nonce_r
                d["cnt"] = cnt0
                return d

            dyn_dummy = {"nonce": None, "cnt": None, "t": 0, "tm1": 0,
                         "t4": 0, "txc": 0}
            # dry pass for register prealloc (collect waits from both)
            if role == 0:
                sp0 = build_sched(0, 0, {}, None)
                sb0 = build_sched(0, None, {}, None)
                sp1 = build_sched(1, 0, {}, None)
                sb1 = build_sched(1, None, {}, None)
                w.prealloc([sp0, sb0, sp1, sb1])

            with eng.If_eq(role_r, role):
                # account init s_g signal once per role branch walk
                if role == 0:
                    w.counts = {s: 0 for s in SEMS}
                    w.counts["s_g"] = 1
                    for s in w.last:
                        w.last[s] = 0
                    # reset thr regs happened at prealloc (reg_mov 0) only once;
                    # role1 branch: re-zero via reg_mov
                else:
                    for s, r in w.regs.items():
                        eng.reg_mov(r, 0)
                    w.counts = {s: 0 for s in SEMS}
                    w.counts["s_g"] = 1
                    for s in w.last:
                        w.last[s] = 0
                if eng_name == "g":
                    dyn_dummy["nonce"] = nonce_r
                    dyn_dummy["cnt"] = cnt0
                peel = build_sched(role, 0, dyn_dummy, eng if eng_name == "g" else None)
                w.walk(peel)
                w.flush()
                if T > 1:
                    with eng.Fori(1, T) as iv:
                        dyn = make_dyn(iv)
                        body = build_sched(role, None, dyn,
                                           eng if eng_name == "g" else None)
                        w.walk(body)
                        w.flush()
                if eng_name == "g" and role == 1:
                    eng.wait_ge(sem_h["s_dg"], cnt0)
                    eng.wait_ge(sem_h["s_pd"], 16 * T)
                    c = 0
                    if lite:
                        eng.dma_start(out=probs_out[:, 0, :],
                                      in_=probs_int[:, T - 1, :]
                                      ).then_inc(sem_h["s_dg"], 16)
                        c += 16
                    else:
                        for b0 in range(0, B, 8):
                            eng.dma_start(out=probs_out[b0:b0 + 8, :, :],
                                          in_=probs_int[b0:b0 + 8, :, :]
                                          ).then_inc(sem_h["s_dg"], 16)
                            c += 16
                    eng.dma_start(out=toks_out[:, :],
                                  in_=toks_int[:, :]).then_inc(sem_h["s_dg"], 16)
                    c += 16
                    eng.reg_add(cnt0, cnt0, c)
                    eng.wait_ge(sem_h["s_dg"], cnt0)

    with nc.Block() as block:
        @block.gpsimd
        def _(g):
            trace_engine("g", g)

        @block.tensor
        def _(p):
            trace_engine("p", p)

        @block.vector
        def _(v):
            trace_engine("v", v)

        @block.scalar
        def _(a):
            trace_engine("a", a)

    nc.finalize()
    return nc


# ================= host side =================

def _pack(inputs, T):
    g = lambda k: np.asarray(inputs[k], np.float32)
    prot = g("protein_embeddings")
    tok0 = np.asarray(inputs["input_token"]).astype(np.int32).reshape(64, 1)
    p1w, p1b = g("proj1_w"), g("proj1_b")
    p2w, p2b = g("proj2_w"), g("proj2_b")
    emb = g("emb")
    sa_w, sa_b = g("sa_w"), g("sa_b")
    ca_w, ca_b = g("ca_w"), g("ca_b")
    w1, b1 = g("ffn_w1"), g("ffn_b1")
    w2, b2 = g("ffn_w2"), g("ffn_b2")
    ln_g, ln_b = g("ln_g"), g("ln_b")
    out_w, out_b = g("out_w"), g("out_b")
    pe = g("pe")

    mem1 = np.maximum(prot @ p1w + p1b, 0.0) @ p2w + p2b
    ca_out = np.stack([(mem1 @ ca_w[l, 2] + ca_b[l, 2]) @ ca_w[l, 3] + ca_b[l, 3]
                       for l in range(L)])
    W_sa = np.stack([sa_w[l, 2] @ sa_w[l, 3] for l in range(L)])
    b_sa = np.stack([sa_b[l, 2] @ sa_w[l, 3] + sa_b[l, 3] for l in range(L)])

    def wmat(l, mat):
        return {"sa": W_sa[l], "w1": w1[l], "w2": w2[l]}[mat]

    def pack_wblk(seq):
        out = np.zeros((128, WCOLS), np.float32)
        cur = 0
        for (l, mat) in seq:
            W = wmat(l, mat)
            nk, nm = _shapes(mat)
            for m in range(nm):
                for k in range(nk):
                    out[:, cur * 128:(cur + 1) * 128] = \
                        W[128 * k:128 * (k + 1), 128 * m:128 * (m + 1)]
                    cur += 1
        return out

    pext = np.zeros((128, 4 * T), np.float32)
    for t in range(T):
        for j in range(NJ):
            pext[:, 4 * t + j] = pe[t, 128 * j:128 * (j + 1)]

    lnp = np.zeros((128, 96), np.float32)
    for l in range(L):
        for ln in range(3):
            for j in range(NJ):
                lnp[:, l * 24 + ln * 8 + 0 + j] = ln_g[l, ln, 128 * j:128 * (j + 1)]
                lnp[:, l * 24 + ln * 8 + 4 + j] = ln_b[l, ln, 128 * j:128 * (j + 1)]

    bsa = np.zeros((128, 16), np.float32)
    b2x = np.zeros((128, 16), np.float32)
    b1x = np.zeros((128, 64), np.float32)
    caxt = np.zeros((128, 1024), np.float32)
    for l in range(L):
        for j in range(NJ):
            bsa[:, l * 4 + j] = b_sa[l, 128 * j:128 * (j + 1)]
            b2x[:, l * 4 + j] = b2[l, 128 * j:128 * (j + 1)]
            for b in range(B):
                caxt[:, l * 256 + j * 64 + b] = ca_out[l, b, 128 * j:128 * (j + 1)]
        for m in range(NM1):
            b1x[:, l * 16 + m] = b1[l, 128 * m:128 * (m + 1)]

    consts = np.zeros((128, 130), np.float32)
    consts[:, 0:128] = np.eye(128, dtype=np.float32)
    consts[:, 128] = 1.0 / 512.0
    consts[:, 129] = EPS
    crow = np.ones((1, 128), np.float32)

    wout = np.zeros((4 * NCHUNK, 128, 512), np.float32)
    for n in range(NCHUNK):
        for k in range(4):
            wout[4 * n + k] = out_w[128 * k:128 * (k + 1), 512 * n:512 * (n + 1)]

    nonce = np.array([[np.random.randint(1, 2 ** 30)]], dtype=np.int32)
    common = dict(
        pext=pext, lnp=lnp, bsa=bsa, b1x=b1x, b2x=b2x, caxt=caxt,
        consts=consts, crow=crow, outb=out_b.reshape(8, 512),
        tok0=tok0, noncei=nonce, emb_hbm=emb, wout_hbm=wout,
    )
    m0 = dict(common, wblk=pack_wblk(SEQ0))
    m1 = dict(common, wblk=pack_wblk(SEQ1))
    return [m0, m1]


def kernel(**inputs):
    from concourse.bass_utils import run_bass_kernel_spmd

    T = T_FULL
    nc = build_program(T)
    in_maps = _pack(inputs, T)
    res = run_bass_kernel_spmd(nc, in_maps, core_ids=[0, 1])
    return np.asarray(res.results[1]["probs_out"], np.float32)

